# revision 28
# baseline (speedup 1.0000x reference)
"""DCNv2 (modulated deformable conv) forward on 8 Trainium2 NeuronCores.

Strategy: data-parallel over batch (B=8, one batch per core).  Per core:
  1. PE GEMM (bf16): z_k[j, oc] = x^T[j, :] @ W_k for the 9 kernel taps,
     written tap-by-tap to a spatial-major DRAM table (rows of 256 oc).
     x columns are host-permuted so 4 consecutive z rows land on one
     partition, letting each tap's z be written with 72 large-descriptor
     DMAs instead of 288.
  2. SWDGE dma_gather: bilinear corners fetched as row PAIRS (r, r+1) — one
     1KB descriptor covers the (x0, x0+1) corner pair of a tap; a second
     (y1-row) descriptor covers the other two corners.  1024-index calls
     balance SWDGE overhead against gather/combine pipelining.
  3. Weighted corner accumulation ON THE PE: for each of the 4 slots a
     diagonal matrix diag(w_slot) is built with one 4x-mode tensor_scalar
     (ident * w), then matmul(psum += diag(w) @ g_slot) accumulates all
     slot terms of a j-tile into PSUM at full fp32 precision.  This keeps
     the DVE off the critical path (a scalar_tensor_tensor combine runs in
     1x mode = 327ns/slot; the diag-build runs 4x = 94ns).
  4. One DVE tensor_tensor fold per (tap-group, j-tile): acc = psum + acc
     (bias seeded), then PE transpose (j,oc)->(oc,j) and DMA out (bf16).

Taps are processed in 3 groups of 3 so PSUM holds only a few j-tile
accumulators at a time while GEMM (group g+1), gathers and combine
(group g) pipeline across engines.

Index/weight prep (floor, fractional weights, border handling) runs on host
in numpy — it is O(B*K*H*W) marshalling, ~0.003% of the conv FLOPs.
"""

from contextlib import ExitStack

import ml_dtypes
import numpy as np

import concourse.bass as bass
import concourse.bacc as bacc
import concourse.mybir as mybir
import concourse.tile as tile
from concourse.bass_utils import run_bass_kernel_spmd
from concourse.masks import make_identity

F32 = mybir.dt.float32
BF16 = mybir.dt.bfloat16
I16 = mybir.dt.int16

# problem constants (hardcoded per harness contract)
B = 8
C = 256
OC = 256
H = W = 64
HW = H * W
K = 9
KH = KW = 3
PAD = 1

T = K
JT = 32            # j-tiles of 128 output positions
CT = 2             # 128-channel contraction tiles
GRP = 3            # taps per pipeline group
NGRP = T // GRP
JTB = 8            # j-tiles per gather block
NBLK = JT // JTB
NIDX = JTB * 128   # idxs per gather call
ZROWS = HW + 1
NQUEUES = 4

_program_cache = {}


def build_program():
    if "nc" in _program_cache:
        return _program_cache["nc"]
    nc = bacc.Bacc("TRN2", target_bir_lowering=False, debug=False,
                   num_swdge_queues=NQUEUES, dynamic_dma_scratch_size=32768)

    x_d = nc.dram_tensor("x", [128, CT, HW], BF16, kind="ExternalInput")
    wr_d = nc.dram_tensor("wr", [128, T, CT, OC], BF16, kind="ExternalInput")
    idx_d = nc.dram_tensor("idx", [128, T, NBLK, 2, NIDX // 16], I16,
                           kind="ExternalInput")
    w4_d = nc.dram_tensor("w4", [128, T, 4, JT], F32, kind="ExternalInput")
    bias_d = nc.dram_tensor("biasb", [128, OC], BF16, kind="ExternalInput")
    out_d = nc.dram_tensor("out", [2, JT // 2, 128, 2, 128], BF16,
                           kind="ExternalOutput")

    with tile.TileContext(nc) as tc, ExitStack() as ctx:
        sp = ctx.enter_context(tc.tile_pool(name="sbuf", bufs=1))
        zst_p = ctx.enter_context(tc.tile_pool(name="zst", bufs=6))
        g_p = ctx.enter_context(tc.tile_pool(name="g", bufs=8))
        dg_p = ctx.enter_context(tc.tile_pool(name="dg", bufs=8))
        acc_p = ctx.enter_context(tc.tile_pool(name="acc", bufs=JT))
        ost_p = ctx.enter_context(tc.tile_pool(name="ost", bufs=4))
        ps_p = ctx.enter_context(tc.tile_pool(name="psum", bufs=2, space="PSUM"))
        po_p = ctx.enter_context(tc.tile_pool(name="psumo", bufs=4, space="PSUM"))
        pst_p = ctx.enter_context(tc.tile_pool(name="psumt", bufs=2, space="PSUM"))
        z_p = ctx.enter_context(tc.tile_pool(name="zdram", bufs=T, space="DRAM"))

        x_sb = sp.tile([128, CT, HW], BF16)
        nc.sync.dma_start(x_sb[:], x_d.ap())
        wr_sb = sp.tile([128, T, CT, OC], BF16)
        nc.sync.dma_start(wr_sb[:], wr_d.ap())
        idx_sb = sp.tile([128, T, NBLK, 2, NIDX // 16], I16)
        nc.sync.dma_start(idx_sb[:], idx_d.ap())
        w4_sb = sp.tile([128, T, 4, JT], F32)
        nc.sync.dma_start(w4_sb[:], w4_d.ap())
        bias_sb = sp.tile([128, OC], BF16)
        nc.sync.dma_start(bias_sb[:], bias_d.ap())
        ident = sp.tile([128, 128], BF16)
        make_identity(nc, ident[:])

        zero_row = sp.tile([1, OC], BF16)
        nc.vector.memset(zero_row[:], 0.0)

        z_tiles = []
        for k in range(T):
            zk = z_p.tile([ZROWS, OC], BF16, name=f"z{k}", tag="z")
            z_tiles.append(zk)
            # slot1 of row HW-1 reads row HW: its weight is always 0, but the
            # value must be finite (0 * NaN = NaN), so zero it.
            nc.sync.dma_start(zk[HW:HW + 1, :], zero_row[:])

        acc = [acc_p.tile([128, OC], BF16, name=f"acc{j}", tag="acc")
               for j in range(JT)]

        gather_state = [0]

        def emit_gemm(grp):
            for k in range(grp * GRP, (grp + 1) * GRP):
                for b in range(JT // 4):
                    zst = zst_p.tile([128, 4, OC], BF16)
                    for q in range(4):
                        ps = ps_p.tile([128, OC], F32)
                        for ct in range(CT):
                            nc.tensor.matmul(
                                ps[:],
                                x_sb[:, ct, (b * 4 + q) * 128:(b * 4 + q + 1) * 128],
                                wr_sb[:, k, ct, :],
                                start=(ct == 0),
                                stop=(ct == CT - 1),
                            )
                        nc.scalar.copy(zst[:, q, :], ps[:])
                    zk_ap = z_tiles[k][:]
                    dst = bass.AP(zk_ap.tensor, zk_ap.offset + b * 512 * OC,
                                  [[4 * OC, 128], [OC, 4], [1, OC]])
                    nc.sync.dma_start(dst, zst[:])

        def emit_combine(grp):
            ks = range(grp * GRP, (grp + 1) * GRP)
            for blk in range(NBLK):
                gs = {}
                for k in ks:
                    zk_ap = z_tiles[k][:]
                    win_ap = bass.AP(zk_ap.tensor, zk_ap.offset,
                                     [[OC, HW], [1, 2 * OC]])
                    for rsel in range(2):
                        g = g_p.tile([128, JTB, 2 * OC], BF16)
                        nc.gpsimd.dma_gather(
                            out_ap=g[:],
                            in_ap=win_ap,
                            idxs_ap=idx_sb[:, k, blk, rsel, :],
                            num_idxs=NIDX,
                            num_idxs_reg=NIDX,
                            elem_size=2 * OC,
                            elem_step=OC,
                            single_packet=False,
                            queue_num=gather_state[0] % NQUEUES,
                        )
                        gather_state[0] += 1
                        gs[k, rsel] = g

                for gi in range(JTB):
                    jt = blk * JTB + gi
                    po = po_p.tile([128, OC], F32)
                    n = 0
                    for k in ks:
                        for rsel in range(2):
                            for sub in range(2):
                                slot = rsel * 2 + sub
                                dg = dg_p.tile([128, 128], BF16)
                                nc.vector.tensor_scalar(
                                    dg[:], ident[:],
                                    w4_sb[:, k, slot, jt:jt + 1], None,
                                    mybir.AluOpType.mult,
                                )
                                nc.tensor.matmul(
                                    po[:],
                                    dg[:],
                                    gs[k, rsel][:, gi, sub * OC:(sub + 1) * OC],
                                    start=(n == 0),
                                    stop=(n == GRP * 4 - 1),
                                )
                                n += 1
                    in1 = bias_sb[:] if grp == 0 else acc[jt][:]
                    nc.vector.tensor_tensor(acc[jt][:], po[:], in1,
                                            mybir.AluOpType.add)

                if grp == NGRP - 1:
                    # phase 4: transpose (j, oc) -> (oc, j); two j-tiles per
                    # store so each partition writes 512B contiguously
                    for jp in range(JTB // 2):
                        for och in range(2):
                            ost = ost_p.tile([128, 2, 128], BF16)
                            for s in range(2):
                                jt = blk * JTB + jp * 2 + s
                                pt = pst_p.tile([128, 128], BF16)
                                nc.tensor.transpose(
                                    pt[:], acc[jt][:, och * 128:(och + 1) * 128],
                                    ident[:],
                                )
                                nc.scalar.copy(ost[:, s, :], pt[:])
                            nc.sync.dma_start(
                                out_d.ap()[och, blk * JTB // 2 + jp], ost[:])

        for grp in range(NGRP):
            emit_gemm(grp)
            emit_combine(grp)

    nc.compile()
    _program_cache["nc"] = nc
    return nc


def _prep_indices_weights(offset_b, mask_b):
    """Per-batch (18,64,64)/(9,64,64) f32 -> pair-base rows r0,r1 (9,4096) and
    slot weights w4 (9,4,4096) with bilinear/mask/validity folded in."""
    off = offset_b.reshape(K, 2, H, W).astype(np.float32)
    m = mask_b.reshape(K, H, W).astype(np.float32)

    oy = np.arange(H, dtype=np.float32) - PAD
    ox = np.arange(W, dtype=np.float32) - PAD
    ky = np.repeat(np.arange(KH, dtype=np.float32), KW)
    kx = np.tile(np.arange(KW, dtype=np.float32), KH)

    py = ky[:, None, None] + oy[None, :, None] + off[:, 0]
    px = kx[:, None, None] + ox[None, None, :] + off[:, 1]

    y0 = np.floor(py)
    x0 = np.floor(px)
    wy = py - y0
    wx = px - x0
    y0i = y0.astype(np.int64)
    x0i = x0.astype(np.int64)

    vy0 = (y0i >= 0) & (y0i < H)
    vy1 = (y0i + 1 >= 0) & (y0i + 1 < H)
    vx0 = (x0i >= 0) & (x0i < W)
    vx1 = (x0i + 1 >= 0) & (x0i + 1 < W)

    w00 = (1 - wy) * (1 - wx) * vy0 * vx0 * m
    w01 = (1 - wy) * wx * vy0 * vx1 * m
    w10 = wy * (1 - wx) * vy1 * vx0 * m
    w11 = wy * wx * vy1 * vx1 * m

    neg_x = x0i < 0
    bx = np.clip(x0i, 0, W - 1)
    s00 = np.where(neg_x, w01, w00)  # x0 == -1: the x0+1 corner sits at slot0
    s01 = np.where(neg_x, 0.0, w01)
    s10 = np.where(neg_x, w11, w10)
    s11 = np.where(neg_x, 0.0, w11)

    yc0 = np.clip(y0i, 0, H - 1)
    yc1 = np.clip(y0i + 1, 0, H - 1)
    r0 = (yc0 * W + bx).reshape(K, HW).astype(np.int32)
    r1 = (yc1 * W + bx).reshape(K, HW).astype(np.int32)

    w4 = np.stack(
        [s00.reshape(K, HW), s01.reshape(K, HW),
         s10.reshape(K, HW), s11.reshape(K, HW)], axis=1
    ).astype(np.float32)
    return r0, r1, w4


def _prep_core_inputs(x_b, offset_b, mask_b, weight, bias):
    r0, r1, w4 = _prep_indices_weights(offset_b, mask_b)

    # permute x columns so GEMM slice q of each 512-row block b produces
    # z rows b*512 + 4p + q on partition p (see build_program phase 1)
    jj = np.arange(HW)
    src = (jj // 512) * 512 + 4 * (jj % 128) + (jj % 512) // 128
    x_in = np.ascontiguousarray(
        x_b.reshape(CT, 128, HW).transpose(1, 0, 2)[:, :, src]
    ).astype(ml_dtypes.bfloat16)

    wk = weight.reshape(OC, CT, 128, K)
    wr = np.ascontiguousarray(wk.transpose(2, 3, 1, 0)).astype(ml_dtypes.bfloat16)

    rs = np.stack([r0, r1], axis=1).reshape(K, 2, NBLK, NIDX)
    rs = rs.transpose(0, 2, 1, 3)  # [k, blk, rsel, i]
    s_idx = np.arange(NIDX // 16)
    p_idx = np.arange(128)
    wrapped = rs[:, :, :, (s_idx[None, :] * 16 + (p_idx[:, None] % 16))]
    idx_in = np.ascontiguousarray(wrapped.transpose(3, 0, 1, 2, 4)).astype(np.int16)

    w4r = w4.reshape(K, 4, JT, 128)
    w4_in = np.ascontiguousarray(w4r.transpose(3, 0, 1, 2)).astype(np.float32)

    bias_in = np.ascontiguousarray(
        np.broadcast_to(bias[None, :], (128, OC))
    ).astype(ml_dtypes.bfloat16)

    return {"x": x_in, "wr": wr, "idx": idx_in, "w4": w4_in, "biasb": bias_in}


def kernel(x, offset, mask, weight, bias):
    x = np.asarray(x, dtype=np.float32)
    offset = np.asarray(offset, dtype=np.float32)
    mask = np.asarray(mask, dtype=np.float32)
    weight = np.asarray(weight, dtype=np.float32)
    bias = np.asarray(bias, dtype=np.float32)

    nc = build_program()
    in_maps = [
        _prep_core_inputs(x[b], offset[b], mask[b], weight, bias)
        for b in range(B)
    ]
    res = run_bass_kernel_spmd(nc, in_maps, core_ids=list(range(B)))

    out = np.empty((B, OC, H, W), dtype=np.float32)
    for b in range(B):
        o = res.results[b]["out"]  # (2, JT//2, 128, 2, 128) bf16
        out[b] = (o.astype(np.float32).transpose(0, 2, 1, 3, 4)
                  .reshape(OC, H, W))
    return out
